# revision 15
# baseline (speedup 1.0000x reference)
"""Trainium2 Bass kernel for nn_DelayExpansionLayer (histogram_binning).

Computation: per-channel mean of layer_output [64,256,56,56] over (B,H,W),
round to 1e-6, nearest-key lookup in a sorted 1024-entry table, max over
channels, scale by (in_ch*out_ch)/512, broadcast to (56,56).

Strategy (data-parallel over batch, 8 NeuronCores):
  - Each core gets 8 batches = [8,256,56,56] (25.7 MB); the host
    TRANSPOSES the shard to partition-major [128, 8, 2, HW] so each
    SBUF partition's whole working set (8 batches x 25KB = 200KB) is
    contiguous in DRAM.
  - Each core computes per-channel partial sums [128, 2] on-device;
    host combines the 8 vectors (the tiny [C] all-reduce) and runs the
    O(C+K) lookup/max/broadcast epilogue.  Channel c = 2*p + j.

Per-core device kernel (raw bass, manual semaphores).  Trace-derived
model: SDMA engine 15 (E79) runs the dynamic queues' bookkeeping; its
deficit vs the other engines' ~26.4 GB/s grows with DMA instruction
count (~0.75us each) and descriptor count (~3ns each).  The classic
batch-major stream needs 2128 descriptors (one per 25KB partition
line), costing E79 ~13us.  Partition-major DRAM lets one descriptor
carry up to 50KB (two batches per partition), so the whole 25.7MB
stream is 7 DMA instructions / ~900 descriptors on ONE queue:
  g0=[b0:2], g1=[b2:4], g2=[b4:6] (2-batch groups, 1x50KB descriptor
  per partition), g3=[b6], then b7 tapered (j0 full, j1 as 1568/1568)
  so the last reduces are short.  All into one 200KB/partition SBUF
  buffer (fits beside the 20KB framework reserve).  Reduces per group:
  DVE tensor_reduce takes j0 (strided [128, nb, HW] -> st1[:, b, 0]),
  ACT activation-Copy accum takes j1 ([128, HW] -> st1[:, b, 1]); two
  tiny final reduces fold st1 [128, 9, 2] into stats [128, 2], which
  leaves via one out-DMA on the (empty) scalar queue.
"""

import sys
import types

import numpy as np

N_CORES = 8
B_FULL, C, H, W = 64, 256, 56, 56
HW = H * W
B_LOCAL = B_FULL // N_CORES
SCALE_DENOM = 32 * 16

# Set by a test harness to enable NTFF tracing of the SPMD run.
TRACE = False
TRACE_TMPDIR = None
LAST_RESULTS = None

_CACHE = {}

# batch groups streamed as single DMAs: (b0, b1)
GROUPS = ((0, 2), (2, 4), (4, 6), (6, 7))
# b7 taper chunks: (j, s0, s1, st1 column)
TAPER = ((0, 0, HW, 7), (1, 0, 1568, 7), (1, 1568, HW, 8))


def _ensure_axon_hooks_shim():
    """bass_utils' axon trace path imports antenv.axon_hooks; provide a
    no-op shim when the environment's antenv package lacks it."""
    try:
        import antenv.axon_hooks  # noqa: F401
        return
    except ImportError:
        pass

    mod = types.ModuleType("antenv.axon_hooks")
    _hook = [None]
    mod.set_axon_ntff_profile_hook = lambda h: _hook.__setitem__(0, h)
    mod.get_axon_ntff_profile_hook = lambda: _hook[0]
    sys.modules["antenv.axon_hooks"] = mod
    try:
        import antenv

        antenv.axon_hooks = mod
    except ImportError:
        pass


def _build():
    if "nc" in _CACHE:
        return _CACHE["nc"]
    import concourse.bass as bass
    from concourse import mybir

    nc = bass.Bass(
        "TRN2",
        target_bir_lowering=False,
        debug=False,
        enable_asserts=False,
        num_devices=N_CORES,
    )
    # Probe: declare only 15 rings on the HW DGE queues so the spray
    # skips engine 15 (the intrinsically ~20% slower bookkeeping engine).
    for q in nc.m.queues:
        if getattr(q, "is_HWDGE", False):
            q.num_queues = 15

    f32 = mybir.dt.float32
    x = nc.dram_tensor(
        "x", [128, B_LOCAL, 2, HW], f32, kind="ExternalInput"
    ).ap()
    out = nc.dram_tensor("out", [128, 2], f32, kind="ExternalOutput").ap()

    big = nc.alloc_sbuf_tensor("big", [128, B_LOCAL, 2, HW], f32).ap()
    # per-batch partial sums: st1[p, b, j]; j0 col 8 unused (final j0
    # reduce reads cols 0:8, j1 reads 0:9)
    st1 = nc.alloc_sbuf_tensor("st1", [128, 9, 2], f32).ap()
    stats = nc.alloc_sbuf_tensor("stats", [128, 2], f32).ap()

    with (
        nc.Block(no_gpsimd_drain=True) as block,
        nc.semaphore("dg0") as dg0,
        nc.semaphore("dg1") as dg1,
        nc.semaphore("dg2") as dg2,
        nc.semaphore("dg3") as dg3,
        nc.semaphore("dt0") as dt0,
        nc.semaphore("dt1") as dt1,
        nc.semaphore("dt2") as dt2,
        nc.semaphore("vd") as vd,
        nc.semaphore("ad") as ad,
        nc.semaphore("od") as od,
    ):
        dg = [dg0, dg1, dg2, dg3]
        dt = [dt0, dt1, dt2]

        @block.sync
        def _(sync: bass.BassEngine):
            # the whole stream: 4 group DMAs + 3 taper chunks, no deps
            for g, (b0, b1) in enumerate(GROUPS):
                sync.dma_start(
                    out=big[:, b0:b1, :, :], in_=x[:, b0:b1, :, :]
                ).then_inc(dg[g], 16)
            for i, (j, s0, s1, _k) in enumerate(TAPER):
                sync.dma_start(
                    out=big[:, 7, j, s0:s1], in_=x[:, 7, j, s0:s1]
                ).then_inc(dt[i], 16)
            sync.wait_ge(od, 16)

        @block.vector
        def _(vector: bass.BassEngine):
            # per-group j0 reduces: [128, nb, HW] -> st1[:, b0:b1, 0]
            for g, (b0, b1) in enumerate(GROUPS):
                vector.wait_ge(dg[g], 16)
                vector.reduce_sum(
                    st1[:, b0:b1, 0:1],
                    big[:, b0:b1, 0, :],
                    axis=mybir.AxisListType.X,
                ).then_inc(vd, 1)
            # taper j0 (b7 col 7)
            vector.wait_ge(dt0, 16)
            vector.reduce_sum(
                st1[:, 7:8, 0:1], big[:, 7, 0, :], axis=mybir.AxisListType.X
            ).then_inc(vd, 1)
            # final folds: j0 over st1 cols 0:8, j1 over 0:9 (j1 cells
            # are ACT's -- wait for its last accumulator writeback)
            vector.reduce_sum(
                stats[:, 0:1], st1[:, 0:8, 0], axis=mybir.AxisListType.X
            ).then_inc(vd, 1)
            vector.wait_ge(ad, 9)
            vector.reduce_sum(
                stats[:, 1:2], st1[:, 0:9, 1], axis=mybir.AxisListType.X
            ).then_inc(vd, 1)

        @block.scalar
        def _(scalar: bass.BassEngine):
            # per-batch j1 reduces via activation-accum
            for g, (b0, b1) in enumerate(GROUPS):
                scalar.wait_ge(dg[g], 16)
                for b in range(b0, b1):
                    scalar.activation(
                        big[:, b, 1, :],
                        big[:, b, 1, :],
                        mybir.ActivationFunctionType.Copy,
                        accum_out=st1[:, b, 1:2],
                    ).then_inc(ad, 1)
            # taper j1 halves (b7 cols 7, 8)
            for i in (1, 2):
                j, s0, s1, k = TAPER[i]
                scalar.wait_ge(dt[i], 16)
                scalar.activation(
                    big[:, 7, 1, s0:s1],
                    big[:, 7, 1, s0:s1],
                    mybir.ActivationFunctionType.Copy,
                    accum_out=st1[:, k, 1:2],
                ).then_inc(ad, 1)
            # single out-DMA on the (empty) scalar queue; vd>=7 is after
            # DVE's final folds (its inc fires once stats is written)
            scalar.wait_ge(vd, 7)
            scalar.dma_start(out=out[:], in_=stats[:]).then_inc(od, 16)

    _CACHE["nc"] = nc
    return nc


def kernel(layer_output, delay_keys, delay_values, in_channels, out_channels):
    global LAST_RESULTS
    _ensure_axon_hooks_shim()
    from concourse.bass_utils import run_bass_kernel_spmd

    x = np.asarray(layer_output, dtype=np.float32)
    assert x.shape == (B_FULL, C, H, W), x.shape
    # shard over batch; channels -> (partition, pair): c = 2*p + j; then
    # transpose each core's shard to partition-major [128, b, j, hw] so a
    # partition's whole working set is contiguous in DRAM (big DMA
    # descriptors -> minimal queue-bookkeeping on SDMA engine 15).
    xr = x.reshape(N_CORES, B_LOCAL, 128, 2, HW).transpose(0, 2, 1, 3, 4)
    in_maps = [{"x": np.ascontiguousarray(xr[k])} for k in range(N_CORES)]

    nc = _build()
    kwargs = {}
    if TRACE:
        kwargs.update(trace=True, tmpdir=TRACE_TMPDIR)
    res = run_bass_kernel_spmd(nc, in_maps, core_ids=list(range(N_CORES)), **kwargs)
    LAST_RESULTS = res

    # tiny [C] all-reduce of the per-core partial sums [128, 2]
    parts = np.stack([res.results[k]["out"] for k in range(N_CORES)])
    sums = parts.sum(axis=0, dtype=np.float32).reshape(C)  # c = 2p+j
    means = sums / np.float32(B_FULL * HW)
    means = np.round(means * np.float32(1e6)) / np.float32(1e6)

    keys = np.asarray(delay_keys, dtype=np.float32)
    values = np.asarray(delay_values, dtype=np.float32)
    K = keys.shape[0]
    idx = np.searchsorted(keys, means)
    lo = np.clip(idx - 1, 0, K - 1)
    hi = np.clip(idx, 0, K - 1)
    pick_hi = np.abs(keys[hi] - means) < np.abs(keys[lo] - means)
    nearest = np.where(pick_hi, hi, lo)
    merged = np.float32(values[nearest].max())

    scale = np.float32(
        (int(np.asarray(in_channels)) * int(np.asarray(out_channels))) / SCALE_DENOM
    )
    return np.full((H, W), merged, dtype=np.float32) * scale


# revision 16
# speedup vs baseline: 1.5088x; 1.5088x over previous
"""Trainium2 Bass kernel for nn_DelayExpansionLayer (histogram_binning).

Computation: per-channel mean of layer_output [64,256,56,56] over (B,H,W),
round to 1e-6, nearest-key lookup in a sorted 1024-entry table, max over
channels, scale by (in_ch*out_ch)/512, broadcast to (56,56).

Strategy (data-parallel over batch, 8 NeuronCores):
  - Each core gets 8 batches = [8,256,56,56] (25.7 MB); the host
    TRANSPOSES the shard to partition-major [128, 8, 2, HW] so each
    SBUF partition's whole working set (8 batches x 25KB = 200KB) is
    contiguous in DRAM.
  - Each core computes per-channel partial sums [128, 2] on-device;
    host combines the 8 vectors (the tiny [C] all-reduce) and runs the
    O(C+K) lookup/max/broadcast epilogue.  Channel c = 2*p + j.

Per-core device kernel (raw bass, manual semaphores).  Trace-derived
model: SDMA engine 15 (E79) runs the dynamic queues' bookkeeping; its
deficit vs the other engines' ~26.4 GB/s grows with DMA instruction
count (~0.75us each) and descriptor count (~3ns each).  The classic
batch-major stream needs 2128 descriptors (one per 25KB partition
line), costing E79 ~13us.  Partition-major DRAM lets one descriptor
carry up to 50KB (two batches per partition), so the whole 25.7MB
stream is 7 DMA instructions / ~900 descriptors on ONE queue:
  g0=[b0:2], g1=[b2:4], g2=[b4:6] (2-batch groups, 1x50KB descriptor
  per partition), g3=[b6], then b7 tapered (j0 full, j1 as 1568/1568)
  so the last reduces are short.  All into one 200KB/partition SBUF
  buffer (fits beside the 20KB framework reserve).  Reduces per group:
  DVE tensor_reduce takes j0 (strided [128, nb, HW] -> st1[:, b, 0]),
  ACT activation-Copy accum takes j1 ([128, HW] -> st1[:, b, 1]); two
  tiny final reduces fold st1 [128, 9, 2] into stats [128, 2], which
  leaves via one out-DMA on the (empty) scalar queue.
"""

import sys
import types

import numpy as np

N_CORES = 8
B_FULL, C, H, W = 64, 256, 56, 56
HW = H * W
B_LOCAL = B_FULL // N_CORES
SCALE_DENOM = 32 * 16

# Set by a test harness to enable NTFF tracing of the SPMD run.
TRACE = False
TRACE_TMPDIR = None
LAST_RESULTS = None

_CACHE = {}

# batch groups streamed as single DMAs: (b0, b1)
GROUPS = ((0, 2), (2, 4), (4, 6), (6, 7))
# b7 taper chunks: (j, s0, s1, st1 column)
TAPER = ((0, 0, HW, 7), (1, 0, 1568, 7), (1, 1568, HW, 8))


def _ensure_axon_hooks_shim():
    """bass_utils' axon trace path imports antenv.axon_hooks; provide a
    no-op shim when the environment's antenv package lacks it."""
    try:
        import antenv.axon_hooks  # noqa: F401
        return
    except ImportError:
        pass

    mod = types.ModuleType("antenv.axon_hooks")
    _hook = [None]
    mod.set_axon_ntff_profile_hook = lambda h: _hook.__setitem__(0, h)
    mod.get_axon_ntff_profile_hook = lambda: _hook[0]
    sys.modules["antenv.axon_hooks"] = mod
    try:
        import antenv

        antenv.axon_hooks = mod
    except ImportError:
        pass


def _build():
    if "nc" in _CACHE:
        return _CACHE["nc"]
    import concourse.bass as bass
    from concourse import mybir

    nc = bass.Bass(
        "TRN2",
        target_bir_lowering=False,
        debug=False,
        enable_asserts=False,
        num_devices=N_CORES,
    )
    f32 = mybir.dt.float32
    x = nc.dram_tensor(
        "x", [128, B_LOCAL, 2, HW], f32, kind="ExternalInput"
    ).ap()
    out = nc.dram_tensor("out", [128, 2], f32, kind="ExternalOutput").ap()

    big = nc.alloc_sbuf_tensor("big", [128, B_LOCAL, 2, HW], f32).ap()
    # per-batch partial sums: st1[p, b, j]; j0 col 8 unused (final j0
    # reduce reads cols 0:8, j1 reads 0:9)
    st1 = nc.alloc_sbuf_tensor("st1", [128, 9, 2], f32).ap()
    stats = nc.alloc_sbuf_tensor("stats", [128, 2], f32).ap()

    with (
        nc.Block(no_gpsimd_drain=True) as block,
        nc.semaphore("dg0") as dg0,
        nc.semaphore("dg1") as dg1,
        nc.semaphore("dg2") as dg2,
        nc.semaphore("dg3") as dg3,
        nc.semaphore("dt0") as dt0,
        nc.semaphore("dt1") as dt1,
        nc.semaphore("dt2") as dt2,
        nc.semaphore("vd") as vd,
        nc.semaphore("ad") as ad,
        nc.semaphore("od") as od,
    ):
        dg = [dg0, dg1, dg2, dg3]
        dt = [dt0, dt1, dt2]

        @block.sync
        def _(sync: bass.BassEngine):
            # the whole stream: 4 group DMAs + 3 taper chunks, no deps
            for g, (b0, b1) in enumerate(GROUPS):
                sync.dma_start(
                    out=big[:, b0:b1, :, :], in_=x[:, b0:b1, :, :]
                ).then_inc(dg[g], 16)
            for i, (j, s0, s1, _k) in enumerate(TAPER):
                sync.dma_start(
                    out=big[:, 7, j, s0:s1], in_=x[:, 7, j, s0:s1]
                ).then_inc(dt[i], 16)
            sync.wait_ge(od, 16)

        @block.vector
        def _(vector: bass.BassEngine):
            # per-group j0 reduces: [128, nb, HW] -> st1[:, b0:b1, 0]
            for g, (b0, b1) in enumerate(GROUPS):
                vector.wait_ge(dg[g], 16)
                vector.reduce_sum(
                    st1[:, b0:b1, 0:1],
                    big[:, b0:b1, 0, :],
                    axis=mybir.AxisListType.X,
                ).then_inc(vd, 1)
            # taper j0 (b7 col 7)
            vector.wait_ge(dt0, 16)
            vector.reduce_sum(
                st1[:, 7:8, 0:1], big[:, 7, 0, :], axis=mybir.AxisListType.X
            ).then_inc(vd, 1)
            # final folds: j0 over st1 cols 0:8, j1 over 0:9 (j1 cells
            # are ACT's -- wait for its last accumulator writeback)
            vector.reduce_sum(
                stats[:, 0:1], st1[:, 0:8, 0], axis=mybir.AxisListType.X
            ).then_inc(vd, 1)
            vector.wait_ge(ad, 9)
            vector.reduce_sum(
                stats[:, 1:2], st1[:, 0:9, 1], axis=mybir.AxisListType.X
            ).then_inc(vd, 1)

        @block.scalar
        def _(scalar: bass.BassEngine):
            # per-batch j1 reduces via activation-accum
            for g, (b0, b1) in enumerate(GROUPS):
                scalar.wait_ge(dg[g], 16)
                for b in range(b0, b1):
                    scalar.activation(
                        big[:, b, 1, :],
                        big[:, b, 1, :],
                        mybir.ActivationFunctionType.Copy,
                        accum_out=st1[:, b, 1:2],
                    ).then_inc(ad, 1)
            # taper j1 halves (b7 cols 7, 8)
            for i in (1, 2):
                j, s0, s1, k = TAPER[i]
                scalar.wait_ge(dt[i], 16)
                scalar.activation(
                    big[:, 7, 1, s0:s1],
                    big[:, 7, 1, s0:s1],
                    mybir.ActivationFunctionType.Copy,
                    accum_out=st1[:, k, 1:2],
                ).then_inc(ad, 1)
            # single out-DMA on the (empty) scalar queue; vd>=7 is after
            # DVE's final folds (its inc fires once stats is written)
            scalar.wait_ge(vd, 7)
            scalar.dma_start(out=out[:], in_=stats[:]).then_inc(od, 16)

    _CACHE["nc"] = nc
    return nc


def kernel(layer_output, delay_keys, delay_values, in_channels, out_channels):
    global LAST_RESULTS
    _ensure_axon_hooks_shim()
    from concourse.bass_utils import run_bass_kernel_spmd

    x = np.asarray(layer_output, dtype=np.float32)
    assert x.shape == (B_FULL, C, H, W), x.shape
    # shard over batch; channels -> (partition, pair): c = 2*p + j; then
    # transpose each core's shard to partition-major [128, b, j, hw] so a
    # partition's whole working set is contiguous in DRAM (big DMA
    # descriptors -> minimal queue-bookkeeping on SDMA engine 15).
    xr = x.reshape(N_CORES, B_LOCAL, 128, 2, HW).transpose(0, 2, 1, 3, 4)
    in_maps = [{"x": np.ascontiguousarray(xr[k])} for k in range(N_CORES)]

    nc = _build()
    kwargs = {}
    if TRACE:
        kwargs.update(trace=True, tmpdir=TRACE_TMPDIR)
    res = run_bass_kernel_spmd(nc, in_maps, core_ids=list(range(N_CORES)), **kwargs)
    LAST_RESULTS = res

    # tiny [C] all-reduce of the per-core partial sums [128, 2]
    parts = np.stack([res.results[k]["out"] for k in range(N_CORES)])
    sums = parts.sum(axis=0, dtype=np.float32).reshape(C)  # c = 2p+j
    means = sums / np.float32(B_FULL * HW)
    means = np.round(means * np.float32(1e6)) / np.float32(1e6)

    keys = np.asarray(delay_keys, dtype=np.float32)
    values = np.asarray(delay_values, dtype=np.float32)
    K = keys.shape[0]
    idx = np.searchsorted(keys, means)
    lo = np.clip(idx - 1, 0, K - 1)
    hi = np.clip(idx, 0, K - 1)
    pick_hi = np.abs(keys[hi] - means) < np.abs(keys[lo] - means)
    nearest = np.where(pick_hi, hi, lo)
    merged = np.float32(values[nearest].max())

    scale = np.float32(
        (int(np.asarray(in_channels)) * int(np.asarray(out_channels))) / SCALE_DENOM
    )
    return np.full((H, W), merged, dtype=np.float32) * scale
